# revision 21
# baseline (speedup 1.0000x reference)
# kernel.py — DeBERTa MoE classifier on 8 Trainium2 NeuronCores (Bass/Tile).
#
# Strategy (data-parallel over batch, 128 samples per core, no collectives):
#   - hidden_states streamed as float8e3 (e3m4: 4 mantissa bits; halves DMA
#     vs fp16 at ~5e-3 final rel err), re-laid-out on host h-chunk-major:
#     x[k][b][s][128]. The x stream is split across BOTH HWDGE rings
#     (sync + scalar) — one ring sustains only ~210 GB/s.
#   - mean-pool via identity-STATIONARY matmuls: the x tile is the moving
#     operand (no LDWEIGHTS in the stream); PSUM accumulates over s with
#     4 interleaved s-subsums per 512-wide matmul, folded+scaled afterwards
#     and PE-transposed into pooledT (fp16) per 128-column chunk.
#   - router/top-k after chunk 0; dense head t1 spread over chunks 3-6
#     (dWT arrives mid-stream); orig head at chunk 7.
#   - tail (pipelined per 512-col segment): e1 matmuls accumulate h1 seg
#     in PSUM -> ACT copy to SBUF + DVE bn_stats -> one batched Sqrt ->
#     LN+gelu fused into ACT Gelu(scale,bias) -> DMA-transpose (X-bar) to
#     gT -> per-expert [HE->C] matmuls into one [128,48] PSUM tile ->
#     weighted combine -> final classifier.
import math
import os
import sys

import numpy as np

for _p in ("/opt/trn_rl_repo", "/root/.axon_site/_ro/trn_rl_repo"):
    if os.path.isdir(_p) and _p not in sys.path:
        sys.path.append(_p)

# Problem dims (hardcoded per spec: nn_DeBERTaMoEClassifier_25374666784925)
B, S, H = 1024, 256, 1024
E, TOPK, HE, C = 16, 4, 256, 3
EPS = 1e-5
N_CORES = 8


class Cfg:
    def __init__(self, b=128, s=S, h=H, e=E, topk=TOPK, he=HE, c=C,
                 ts=128, dt_x="float8e3", dt_w="float16", dve_chunk=2):
        self.b, self.s, self.h, self.e, self.topk, self.he, self.c = b, s, h, e, topk, he, c
        self.ts = ts                      # s-positions per stream tile
        assert s % self.ts == 0
        assert h % 128 == 0 and b == 128
        self.eo = e * he
        self.dt_x = dt_x
        self.dt_w = dt_w
        self.dve_chunk = dve_chunk   # h-chunk pooled on DVE (None = all on PE)
        self.dr_chunks = (5, 6, 7)   # h-chunks streamed e4m3 + DoubleRow-pooled


def _np_dt(name):
    import ml_dtypes
    return {"float16": np.float16, "float8e3": ml_dtypes.float8_e3m4,
            "float8e4": ml_dtypes.float8_e4m3, "float32": np.float32}[name]


def host_prep(inputs, cfg):
    """Split/transpose/cast inputs on the host. Returns (shared, per_core, flags)."""
    f32 = np.float32
    dtw = _np_dt(cfg.dt_w)
    dtx = _np_dt(cfg.dt_x)
    hs = np.asarray(inputs["hidden_states"], dtype=f32)
    nb = hs.shape[0] // cfg.b  # number of cores
    hch = cfg.h // 128

    eW1 = np.asarray(inputs["eW1"], f32)     # [E, HE, H]
    eW2 = np.asarray(inputs["eW2"], f32)     # [E, HE, HE]
    proj_W = np.asarray(inputs["proj_W"], f32)   # [C, HE]
    dense_W = np.asarray(inputs["dense_W"], f32)  # [H, H] (out, in)
    router_W = np.asarray(inputs["router_W"], f32)  # [E, H]
    out_W = np.asarray(inputs["out_W"], f32)  # [C, H]
    f1_W = np.asarray(inputs["f1_W"], f32)    # [C, 2C]
    f2_W = np.asarray(inputs["f2_W"], f32)    # [C, C]

    W2P = np.einsum("co,eoh->ech", proj_W, eW2)          # [E, C, HE]
    B2P = proj_W @ np.asarray(inputs["eb2"], f32).T      # [C, E]
    B2P = (B2P.T + np.asarray(inputs["proj_b"], f32)[None, :])  # [E, C]

    def img(arr2d, dt):
        # [K*128, W] -> [128, K*W] partition-major SBUF image (contiguous DMA)
        k = arr2d.shape[0] // 128
        return np.ascontiguousarray(
            arr2d.reshape(k, 128, -1).transpose(1, 0, 2).reshape(128, -1)).astype(dt)

    # e1T image: [128, hch, EO], e1T[p, k, n] = eW1[e, he, k*128+p] with n=e*HE+he
    e1T = img(eW1.transpose(2, 0, 1).reshape(cfg.h, cfg.eo), dtw)
    # w2pT image: [128, E*2*C]; w2pT[p, (e*2+j)*C+c] = W2P[e, c, j*128+p]
    kch = cfg.he // 128
    w2pT = np.ascontiguousarray(
        W2P.reshape(cfg.e, cfg.c, kch, 128).transpose(3, 0, 2, 1)
        .reshape(128, cfg.e * kch * cfg.c)).astype(dtw)

    shared = {
        "e1T": e1T,
        "dWT": img(dense_W.T, dtw),
        "rWT": img(router_W.T, f32),
        "oWT": img(out_W.T, dtw),
        "w2pT": w2pT,
        "id32": np.eye(128, dtype=f32),
        "idx": np.eye(128).astype(dtx),
        "id2": np.concatenate([np.eye(128), np.eye(128)], axis=1)
               .astype(_np_dt("float8e4")),
    }
    for i in range(2 * cfg.c):
        shared[f"f1row{i}"] = np.ascontiguousarray(f1_W.T[i:i + 1, :])  # [1, C]
    for i in range(cfg.c):
        shared[f"f2row{i}"] = np.ascontiguousarray(f2_W.T[i:i + 1, :])  # [1, C]

    flags = {}
    flags["f1T_vals"] = f1_W.T.tolist()        # [2C][C]
    flags["f2T_vals"] = f2_W.T.tolist()        # [C][C]
    flags["f1b_vals"] = np.asarray(inputs["f1_b"], f32).tolist()
    flags["f2b_vals"] = np.asarray(inputs["f2_b"], f32).tolist()
    flags["fg_vals"] = np.asarray(inputs["fg"], f32).tolist()
    flags["fbt_vals"] = np.asarray(inputs["fbt"], f32).tolist()

    def nz(key):
        v = np.asarray(inputs[key], f32)
        return bool(np.any(v != 0.0))

    flags["router_b"] = nz("router_b")
    flags["eb1"] = nz("eb1")
    flags["eg_ebt"] = bool(np.any(np.asarray(inputs["eg"], f32) != 1.0)) or nz("ebt")
    flags["b2p"] = bool(np.any(B2P != 0.0))
    flags["dense_b"] = nz("dense_b")
    flags["out_b"] = nz("out_b")
    flags["f1_b"] = nz("f1_b")
    flags["fg_fbt"] = bool(np.any(np.asarray(inputs["fg"], f32) != 1.0)) or nz("fbt")
    flags["f2_b"] = nz("f2_b")
    need_ones16 = flags["eb1"]
    need_ones32 = (flags["router_b"] or flags["b2p"] or flags["out_b"]
                   or flags["f1_b"] or flags["f2_b"])
    if need_ones16:
        shared["ones16"] = np.ones((1, 128), dtype=dtw)
        shared["eb1row"] = np.asarray(inputs["eb1"], f32).reshape(1, cfg.eo).astype(dtw)
    if need_ones32:
        shared["ones32"] = np.ones((1, 128), dtype=f32)
    if flags["router_b"]:
        shared["rb32"] = np.asarray(inputs["router_b"], f32).reshape(1, cfg.e)
    if flags["b2p"]:
        shared["b2prow"] = np.ascontiguousarray(B2P.reshape(1, cfg.e * cfg.c))
    if flags["out_b"]:
        shared["outb32"] = np.asarray(inputs["out_b"], f32).reshape(1, cfg.c)
    if flags["f1_b"]:
        shared["f1b32"] = np.asarray(inputs["f1_b"], f32).reshape(1, cfg.c)
    if flags["f2_b"]:
        shared["f2b32"] = np.asarray(inputs["f2_b"], f32).reshape(1, cfg.c)
    if flags["dense_b"]:
        shared["db2"] = np.ascontiguousarray(
            np.asarray(inputs["dense_b"], f32).reshape(hch, 128).T)  # [128, hch]
    if flags["eg_ebt"]:
        shared["egrow"] = np.asarray(inputs["eg"], f32).reshape(1, cfg.eo)
        shared["ebtrow"] = np.asarray(inputs["ebt"], f32).reshape(1, cfg.eo)

    # x cast per h-chunk group (e3m4 base, e4m3 for DoubleRow chunks),
    # then per-core h-chunk-major relayout x[k][b][s][128] contiguous
    dtx4 = _np_dt("float8e4")
    e3_chunks = [k for k in range(hch) if k not in cfg.dr_chunks]
    per_core = []
    for ci in range(nb):
        hc = hs[ci * cfg.b:(ci + 1) * cfg.b]               # [128, S, H] f32
        hc4 = hc.reshape(cfg.b, cfg.s, hch, 128)
        xr = np.ascontiguousarray(
            hc4[:, :, e3_chunks, :].transpose(2, 0, 1, 3)).astype(dtx)
        xr4 = np.ascontiguousarray(
            hc4[:, :, list(cfg.dr_chunks), :].transpose(2, 0, 1, 3)).astype(dtx4)
        clsT = hc[:, 0, :].T                               # [H, 128] f32
        per_core.append({
            "x": xr,
            "x4": xr4,
            "clsT32": img(clsT, f32),
            "clsT16": img(clsT, dtw),
        })
    return shared, per_core, flags


def build_program(nc, tc, ctx, cfg, flags, debug=False):
    """Emit the whole per-core program inside TileContext `tc`."""
    import concourse.bass as bass
    import concourse.mybir as mybir
    import concourse.tile as tile

    f32 = mybir.dt.float32
    dtw = getattr(mybir.dt, cfg.dt_w)
    dtx = getattr(mybir.dt, cfg.dt_x)
    AF = mybir.ActivationFunctionType
    OP = mybir.AluOpType
    AX = mybir.AxisListType

    b, s, h, e, he, c, eo = cfg.b, cfg.s, cfg.h, cfg.e, cfg.he, cfg.c, cfg.eo
    ts = cfg.ts
    hch = h // 128
    n_t = s // ts            # stream tiles per h-chunk (2)
    mm_s = 4                 # s-positions per pooling matmul (N = 512)
    kch = he // 128          # he chunks per expert (2)
    n_seg = eo // 512        # h1 segments (8)
    ng = 512 // he           # LN groups per segment (2)

    # ---- DRAM tensors -------------------------------------------------
    def din(name, shape, dt):
        return nc.dram_tensor(name, list(shape), dt, kind="ExternalInput").ap()

    dtx4 = mybir.dt.float8e4
    e3_chunks = [k for k in range(hch) if k not in cfg.dr_chunks]
    x_d = din("x", [len(e3_chunks), b, s, 128], dtx)
    x4_d = din("x4", [len(cfg.dr_chunks), b, s, 128], dtx4)
    chunk_src = {}
    for i, k in enumerate(e3_chunks):
        chunk_src[k] = x_d[i]
    for i, k in enumerate(cfg.dr_chunks):
        chunk_src[k] = x4_d[i]
    clsT32_d = din("clsT32", [128, hch * b], f32)
    clsT16_d = din("clsT16", [128, hch * b], dtw)
    e1T_d = din("e1T", [128, hch * eo], dtw)
    dWT_d = din("dWT", [128, hch * h], dtw)
    rWT_d = din("rWT", [128, hch * e], f32)
    oWT_d = din("oWT", [128, hch * c], dtw)
    w2pT_d = din("w2pT", [128, e * kch * c], dtw)
    id32_d = din("id32", [128, 128], f32)
    idx_d = din("idx", [128, 128], dtx)
    id2_d = din("id2", [128, 256], dtx4)
    f1row_d = [din(f"f1row{i}", [1, c], f32) for i in range(2 * c)]
    f2row_d = [din(f"f2row{i}", [1, c], f32) for i in range(c)]
    opt_d = {}
    for key, shape, dt in [
        ("ones16", (1, 128), dtw), ("eb1row", (1, eo), dtw),
        ("ones32", (1, 128), f32), ("rb32", (1, e), f32),
        ("b2prow", (1, e * c), f32), ("outb32", (1, c), f32),
        ("f1b32", (1, c), f32), ("f2b32", (1, c), f32),
        ("db2", (128, hch), f32), ("egrow", (1, eo), f32),
        ("ebtrow", (1, eo), f32),
    ]:
        need = {
            "ones16": flags["eb1"], "eb1row": flags["eb1"],
            "ones32": (flags["router_b"] or flags["b2p"] or flags["out_b"]
                       or flags["f1_b"] or flags["f2_b"]),
            "rb32": flags["router_b"], "b2prow": flags["b2p"],
            "outb32": flags["out_b"], "f1b32": flags["f1_b"],
            "f2b32": flags["f2_b"], "db2": flags["dense_b"],
            "egrow": flags["eg_ebt"], "ebtrow": flags["eg_ebt"],
        }[key]
        if need:
            opt_d[key] = din(key, shape, dt)

    out_d = nc.dram_tensor("out", [b, c], f32, kind="ExternalOutput").ap()
    dbg = {}
    if debug:
        for name, shape in [("dbg_logits", [b, e]), ("dbg_pooledT", [hch, 128, b]),
                            ("dbg_h1", [b, eo]), ("dbg_comb", [b, 2 * c])]:
            dbg[name] = nc.dram_tensor(name, shape, f32, kind="ExternalOutput").ap()

    # ---- pools --------------------------------------------------------
    const = ctx.enter_context(tc.tile_pool(name="const", bufs=1))
    xpool = ctx.enter_context(tc.tile_pool(name="xpool", bufs=3))
    xpool0 = ctx.enter_context(tc.tile_pool(name="xpool0", bufs=2))
    work = ctx.enter_context(tc.tile_pool(name="work", bufs=2))
    small = ctx.enter_context(tc.tile_pool(name="small", bufs=1))
    # PSUM budget (8 banks): pool 2 + mm 2 + t1/el 2 + pssm 2
    pool_psum = ctx.enter_context(tc.tile_pool(name="pool_psum", bufs=2, space="PSUM"))
    mm_psum = ctx.enter_context(tc.tile_pool(name="mm_psum", bufs=2, space="PSUM"))
    t1_psum = ctx.enter_context(tc.tile_pool(name="t1_psum", bufs=1, space="PSUM"))
    tr_psum = ctx.enter_context(tc.tile_pool(name="tr_psum", bufs=2, space="PSUM"))

    # ---- early consts on the scalar ring ------------------------------
    idx_sb = const.tile([128, 128], dtx)
    nc.scalar.dma_start(out=idx_sb, in_=idx_d)
    id2_sb = const.tile([128, 2, 128], dtx4)
    nc.scalar.dma_start(out=id2_sb, in_=id2_d.rearrange("p (j m) -> p j m", j=2))
    f1bc = const.tile([128, 2 * c, c], f32)
    for i in range(2 * c):
        nc.scalar.dma_start(out=f1bc[:, i, :],
                            in_=f1row_d[i].to_broadcast((128, c)))
    f2bc = const.tile([128, c, c], f32)
    for i in range(c):
        nc.scalar.dma_start(out=f2bc[:, i, :],
                            in_=f2row_d[i].to_broadcast((128, c)))
    id32_sb = const.tile([128, 128], f32)
    nc.scalar.dma_start(out=id32_sb, in_=id32_d)
    clsT32_sb = const.tile([128, hch, b], f32)
    nc.scalar.dma_start(out=clsT32_sb, in_=clsT32_d.rearrange("p (k b) -> p k b", k=hch))
    clsT16_sb = const.tile([128, hch, b], dtw)
    nc.scalar.dma_start(out=clsT16_sb, in_=clsT16_d.rearrange("p (k b) -> p k b", k=hch))
    rWT_sb = const.tile([128, hch, e], f32)
    nc.scalar.dma_start(out=rWT_sb, in_=rWT_d.rearrange("p (k e) -> p k e", k=hch))
    opt_sb = {}
    for key, ap in opt_d.items():
        t = const.tile(list(ap.shape), ap.dtype, name=f"{key}_sb")
        nc.scalar.dma_start(out=t, in_=ap)
        opt_sb[key] = t
    eps_sb = const.tile([128, 1], f32)
    nc.vector.memset(eps_sb, EPS)

    # tiles for late consts (DMAs interleaved into the stream below)
    e1T_sb = const.tile([128, hch, eo], dtw)
    e1T_r = e1T_d.rearrange("p (k n) -> p k n", k=hch)
    dWT_sb = const.tile([128, hch, h], dtw)
    oWT_sb = const.tile([128, hch, c], dtw)
    w2pT_sb = const.tile([128, e * kch, c], dtw)

    # ---- persistent SBUF state ---------------------------------------
    pooledT_sb = const.tile([128, hch, b], dtw, name="pooledT_sb")
    t1T_sb = const.tile([128, hch, b], dtw, name="t1T_sb")
    h1s = const.tile([128, eo], f32, name="h1s")
    gT_sb = const.tile([128, eo // 128, b], dtw, name="gT_sb")
    comb_sb = small.tile([128, 2 * c], f32)
    t1acc = t1_psum.tile([128, hch, b], f32, name="t1acc", tag="t1el")

    # ---- stream -------------------------------------------------------
    # x tile i=2k+t rides the scalar ring for i in SCALAR_SET, else sync;
    # late consts are interleaved on the scalar ring to balance both rings.
    def emit_late_consts(i):
        if i == 3:
            nc.scalar.dma_start(out=dWT_sb,
                                in_=dWT_d.rearrange("p (k o) -> p k o", k=hch))
        elif i == 7:
            nc.scalar.dma_start(out=oWT_sb,
                                in_=oWT_d.rearrange("p (k c) -> p k c", k=hch))
            nc.scalar.dma_start(
                out=w2pT_sb, in_=w2pT_d.rearrange("p (g c) -> p g c", g=e * kch))

    acc4 = const.tile([128, mm_s * 128], f32, name="acc4")
    nc.vector.memset(acc4, 0.0)

    def emit_pool_chunk(k):
        # one h-chunk on DVE; dr_chunks via e4m3 DoubleRow (2 s-pos/pass)
        on_dve = (k == cfg.dve_chunk)
        on_dr = (k in cfg.dr_chunks)
        if on_dve:
            pp = None
        elif on_dr:
            pp = pool_psum.tile([128, 128], f32, name="ppd", tag="poolacc")
        else:
            pp = pool_psum.tile([128, mm_s * 128], f32, name="pp", tag="poolacc")
        n_mm = ts // mm_s
        for t in range(n_t):
            i = 2 * k + t
            pool = xpool0 if on_dve else xpool
            xt = pool.tile([128, ts, 128], dtx4 if on_dr else dtx, name="xt",
                           tag="xts")
            eng = nc.sync if t == 0 else nc.scalar
            eng.dma_start(out=xt, in_=chunk_src[k][:, t * ts:(t + 1) * ts, :])
            if t == 0 and k >= 4:
                kk = k - 4
                nc.sync.dma_start(out=e1T_sb[:, kk, :], in_=e1T_r[:, kk, :])
            if t == 1 and k >= 4:
                nc.scalar.dma_start(out=e1T_sb[:, k, :], in_=e1T_r[:, k, :])
            emit_late_consts(i)
            if on_dr:
                for p in range(ts // 2):
                    nc.tensor.matmul(
                        pp, id2_sb, xt[:, 2 * p:2 * p + 2, :],
                        start=(t == 0 and p == 0),
                        stop=(t == n_t - 1 and p == ts // 2 - 1),
                        perf_mode=mybir.MatmulPerfMode.DoubleRow)
            else:
                for j in range(n_mm):
                    if on_dve:
                        nc.vector.tensor_add(acc4, acc4,
                                             xt[:, j * mm_s:(j + 1) * mm_s, :])
                    else:
                        nc.tensor.matmul(
                            pp, idx_sb, xt[:, j * mm_s:(j + 1) * mm_s, :],
                            start=(t == 0 and j == 0),
                            stop=(t == n_t - 1 and j == n_mm - 1))
        return pp

    def emit_router_topk():
        logits_ps = tr_psum.tile([128, e], f32, name="logits_ps", tag="pssm")
        for k in range(hch):
            nc.tensor.matmul(logits_ps, clsT32_sb[:, k, :], rWT_sb[:, k, :],
                             start=(k == 0),
                             stop=(k == hch - 1 and not flags["router_b"]))
        if flags["router_b"]:
            nc.tensor.matmul(logits_ps, opt_sb["ones32"], opt_sb["rb32"],
                             start=False, stop=True)
        L_sb = small.tile([128, e], f32)
        nc.vector.tensor_copy(L_sb, logits_ps)
        if debug:
            nc.sync.dma_start(out=dbg["dbg_logits"], in_=L_sb)
        m1 = small.tile([128, 1], f32)
        nc.vector.reduce_max(m1, L_sb, axis=AX.X)
        negm1 = small.tile([128, 1], f32)
        nc.vector.tensor_scalar_mul(negm1, m1, -1.0)
        eall = small.tile([128, e], f32)
        nc.scalar.activation(out=eall, in_=L_sb, func=AF.Exp, bias=negm1, scale=1.0)
        lcur = L_sb
        mk = m1
        for kk in range(cfg.topk - 1):
            eq = small.tile([128, e], f32, name=f"eq{kk}")
            nc.vector.tensor_scalar(eq, lcur, mk, None, op0=OP.is_equal)
            lnext = small.tile([128, e], f32, name=f"lnext{kk}")
            nc.vector.scalar_tensor_tensor(out=lnext, in0=eq, scalar=-1e30, in1=lcur,
                                           op0=OP.mult, op1=OP.add)
            mk = small.tile([128, 1], f32, name=f"mk{kk}")
            nc.vector.reduce_max(mk, lnext, axis=AX.X)
            lcur = lnext
        mask = small.tile([128, e], f32)
        nc.vector.tensor_scalar(mask, L_sb, mk, None, op0=OP.is_ge)
        wu = small.tile([128, e], f32)
        nc.vector.tensor_mul(wu, eall, mask)
        den = small.tile([128, 1], f32)
        nc.vector.reduce_sum(den, wu, axis=AX.X)
        winv = small.tile([128, 1], f32)
        nc.vector.reciprocal(winv, den)
        return wu, winv

    def emit_t1_quarter(q):
        # 16 dense-head matmuls (ko = 2q, 2q+1); start once per 2KB region
        for ko in (2 * q, 2 * q + 1):
            for k in range(hch):
                nc.tensor.matmul(t1acc[:, ko, :], dWT_sb[:, k, bass.ts(ko, 128)],
                                 clsT16_sb[:, k, :],
                                 start=(k == 0 and ko % 4 == 0),
                                 stop=(k == hch - 1 and ko % 4 == 3))
        if q in (1, 3):   # region complete -> tanh evacuation
            for ko in range(4 * (q // 2), 4 * (q // 2) + 4):
                if flags["dense_b"]:
                    nc.scalar.activation(out=t1T_sb[:, ko, :], in_=t1acc[:, ko, :],
                                         func=AF.Tanh,
                                         bias=opt_sb["db2"][:, ko:ko + 1], scale=1.0)
                else:
                    nc.scalar.activation(out=t1T_sb[:, ko, :], in_=t1acc[:, ko, :],
                                         func=AF.Tanh)

    def emit_orig():
        orig_ps = tr_psum.tile([128, c], f32, name="orig_ps", tag="pssm")
        for k in range(hch):
            nc.tensor.matmul(orig_ps, t1T_sb[:, k, :], oWT_sb[:, k, :],
                             start=(k == 0),
                             stop=(k == hch - 1 and not flags["out_b"]))
        if flags["out_b"]:
            nc.tensor.matmul(orig_ps, opt_sb["ones32"], opt_sb["outb32"],
                             start=False, stop=True)
        nc.vector.tensor_copy(comb_sb[:, 0:c], orig_ps)

    def emit_chunk_epilogue(k, pp):
        # fold subsums + scale by 1/S -> f32 SBUF; ACT starts, DVE chains
        src_t = acc4 if pp is None else pp
        nfold = 1 if (pp is not None and k in cfg.dr_chunks) else mm_s
        u = work.tile([128, 128], f32, name="u", tag="ufold")
        nc.scalar.activation(out=u, in_=src_t[:, 0:128], func=AF.Copy,
                             scale=1.0 / float(s))
        for j in range(1, nfold):
            nc.vector.scalar_tensor_tensor(out=u, in0=src_t[:, j * 128:(j + 1) * 128],
                                           scalar=1.0 / float(s), in1=u,
                                           op0=OP.mult, op1=OP.add)
        uT_ps = tr_psum.tile([128, b], f32, name="uT_ps", tag="pssm")
        nc.tensor.transpose(uT_ps, u, id32_sb)
        nc.scalar.activation(out=pooledT_sb[:, k, :], in_=uT_ps, func=AF.Copy)
        # extra per-chunk work, spread across the stream
        if k == 0:
            state["wu"], state["winv"] = emit_router_topk()
        elif 3 <= k <= 6:
            emit_t1_quarter(k - 3)
        elif k == 7:
            emit_orig()

    state = {}
    prev = None
    for k in range(hch):
        pp = emit_pool_chunk(k)
        if k > 0:
            emit_chunk_epilogue(k - 1, prev)
        prev = pp
    emit_chunk_epilogue(hch - 1, prev)
    wu, winv = state["wu"], state["winv"]

    if debug:
        pooledT32 = small.tile([128, hch, b], f32, name="pooledT32")
        nc.vector.tensor_copy(pooledT32, pooledT_sb)
        nc.sync.dma_start(out=dbg["dbg_pooledT"].rearrange("k p b -> p k b"),
                          in_=pooledT32)

    # ---- tail ---------------------------------------------------------
    # per segment: e1 matmuls -> PSUM; ACT copy -> h1s; DVE bn_stats
    mv = small.tile([128, e, 2], f32, name="mv")
    for g in range(n_seg):
        hp = mm_psum.tile([128, 512], f32, name="hp", tag="mmq")
        for k in range(hch):
            nc.tensor.matmul(hp, pooledT_sb[:, k, :],
                             e1T_sb[:, k, g * 512:(g + 1) * 512],
                             start=(k == 0),
                             stop=(k == hch - 1 and not flags["eb1"]))
        if flags["eb1"]:
            nc.tensor.matmul(hp, opt_sb["ones16"],
                             opt_sb["eb1row"][:, g * 512:(g + 1) * 512],
                             start=False, stop=True)
        nc.scalar.activation(out=h1s[:, g * 512:(g + 1) * 512], in_=hp, func=AF.Copy)
        for q in range(ng):
            st = work.tile([128, 6], f32, name="st")
            nc.vector.bn_stats(out=st, in_=hp[:, q * he:(q + 1) * he])
            nc.vector.bn_aggr(out=mv[:, g * ng + q, :], in_=st)
    if debug:
        nc.sync.dma_start(out=dbg["dbg_h1"], in_=h1s)

    # batched LN scalars: rstd, bias = -mean*rstd
    sd = small.tile([128, e], f32)
    nc.scalar.activation(out=sd, in_=mv[:, :, 1], func=AF.Sqrt, bias=eps_sb, scale=1.0)
    rstd = small.tile([128, e], f32)
    nc.vector.reciprocal(rstd, sd)
    nb = small.tile([128, e], f32)
    nc.vector.tensor_mul(nb, mv[:, :, 0], rstd)
    nc.vector.tensor_scalar_mul(nb, nb, -1.0)

    # LN+gelu fused on ACT (per he-group, in place on h1s); then per
    # 128-chunk: PE transpose -> evac (ACT/DVE alternating) -> el matmul.
    if flags["eg_ebt"]:
        for q in range(e):
            nc.vector.tensor_scalar(h1s[:, q * he:(q + 1) * he],
                                    h1s[:, q * he:(q + 1) * he],
                                    mv[:, q, 0:1], rstd[:, q:q + 1],
                                    op0=OP.subtract, op1=OP.mult)
        eg_sb = work.tile([128, eo], f32, name="eg_sb", tag="egb")
        nc.sync.dma_start(out=eg_sb, in_=opt_d["egrow"].to_broadcast((128, eo)))
        ebt_sb = work.tile([128, eo], f32, name="ebt_sb", tag="egb")
        nc.sync.dma_start(out=ebt_sb, in_=opt_d["ebtrow"].to_broadcast((128, eo)))
        nc.vector.tensor_mul(h1s, h1s, eg_sb)
        nc.vector.tensor_add(h1s, h1s, ebt_sb)
        nc.scalar.activation(out=h1s, in_=h1s, func=AF.Gelu)
    el_ps = t1_psum.tile([128, e * c], f32, name="el_ps", tag="t1el")
    n_ch = eo // 128

    def emit_el(gi):
        ei = gi // kch
        nc.tensor.matmul(el_ps[:, ei * c:(ei + 1) * c], gT_sb[:, gi, :],
                         w2pT_sb[:, gi, :],
                         start=(gi == 0),
                         stop=(gi == n_ch - 1 and not flags["b2p"]))

    for q in range(e):
        if not flags["eg_ebt"]:
            nc.scalar.activation(out=h1s[:, q * he:(q + 1) * he],
                                 in_=h1s[:, q * he:(q + 1) * he], func=AF.Gelu,
                                 scale=rstd[:, q:q + 1], bias=nb[:, q:q + 1])
        for gi in (2 * q, 2 * q + 1):
            nT_ps = tr_psum.tile([128, b], f32, name="nT_ps", tag="pssm")
            nc.tensor.transpose(nT_ps, h1s[:, gi * 128:(gi + 1) * 128], id32_sb)
            if gi % 2 == 0:
                nc.scalar.activation(out=gT_sb[:, gi, :], in_=nT_ps, func=AF.Copy)
            else:
                nc.vector.tensor_copy(gT_sb[:, gi, :], nT_ps)
        if q > 0:
            emit_el(2 * q - 2)
            emit_el(2 * q - 1)
    emit_el(n_ch - 2)
    emit_el(n_ch - 1)
    if flags["b2p"]:
        nc.tensor.matmul(el_ps, opt_sb["ones32"], opt_sb["b2prow"],
                         start=False, stop=True)

    # weighted combine (4 parallel chains + tree fold):
    # moe = sum_e wu[:, e] * el[:, e*c:(e+1)*c]
    mp = small.tile([128, 4, c], f32, name="mp")
    for lane in range(4):
        e0 = lane * 4
        nc.vector.tensor_scalar_mul(mp[:, lane, :], el_ps[:, e0 * c:(e0 + 1) * c],
                                    wu[:, e0:e0 + 1])
        for ei in range(e0 + 1, e0 + 4):
            nc.vector.scalar_tensor_tensor(out=mp[:, lane, :],
                                           in0=el_ps[:, ei * c:(ei + 1) * c],
                                           scalar=wu[:, ei:ei + 1],
                                           in1=mp[:, lane, :],
                                           op0=OP.mult, op1=OP.add)
    m01 = small.tile([128, c], f32)
    nc.vector.tensor_add(m01, mp[:, 0, :], mp[:, 1, :])
    m23 = small.tile([128, c], f32)
    nc.vector.tensor_add(m23, mp[:, 2, :], mp[:, 3, :])
    macc = small.tile([128, c], f32)
    nc.vector.tensor_add(macc, m01, m23)
    nc.vector.tensor_scalar_mul(comb_sb[:, c:2 * c], macc, winv)
    if debug:
        nc.sync.dma_start(out=dbg["dbg_comb"], in_=comb_sb)

    # ---- final classifier (DVE, wide ops over broadcast weight rows) ---
    t_sb = small.tile([128, c], f32)
    nc.vector.tensor_scalar_mul(t_sb, f1bc[:, 0, :], comb_sb[:, 0:1])
    for i in range(1, 2 * c):
        nc.vector.scalar_tensor_tensor(out=t_sb, in0=f1bc[:, i, :],
                                       scalar=comb_sb[:, i:i + 1], in1=t_sb,
                                       op0=OP.mult, op1=OP.add)
    if flags["f1_b"]:
        for j in range(c):
            nc.vector.tensor_single_scalar(out=t_sb[:, j:j + 1],
                                           in_=t_sb[:, j:j + 1],
                                           scalar=float(flags["f1b_vals"][j]),
                                           op=OP.add)
    # LN over c elements (manual; c is odd)
    msum = small.tile([128, 1], f32)
    nc.vector.reduce_sum(msum, t_sb, axis=AX.X)
    mf = small.tile([128, 1], f32)
    nc.vector.tensor_scalar_mul(mf, msum, 1.0 / float(c))
    ctr = small.tile([128, c], f32)
    nc.vector.tensor_scalar(ctr, t_sb, mf, None, op0=OP.subtract)
    sq = small.tile([128, c], f32)
    nc.vector.tensor_mul(sq, ctr, ctr)
    vsum = small.tile([128, 1], f32)
    nc.vector.reduce_sum(vsum, sq, axis=AX.X)
    sdf = small.tile([128, 1], f32)
    nc.scalar.activation(out=sdf, in_=vsum, func=AF.Sqrt, bias=eps_sb,
                         scale=1.0 / float(c))
    rstdf = small.tile([128, 1], f32)
    nc.vector.reciprocal(rstdf, sdf)
    z_sb = small.tile([128, c], f32)
    nc.vector.tensor_scalar_mul(z_sb, ctr, rstdf)
    if flags["fg_fbt"]:
        for j in range(c):
            nc.vector.tensor_scalar_mul(z_sb[:, j:j + 1], z_sb[:, j:j + 1],
                                        float(flags["fg_vals"][j]))
            nc.vector.tensor_single_scalar(out=z_sb[:, j:j + 1],
                                           in_=z_sb[:, j:j + 1],
                                           scalar=float(flags["fbt_vals"][j]),
                                           op=OP.add)
    nc.vector.tensor_single_scalar(out=z_sb, in_=z_sb, scalar=0.0, op=OP.max)
    out_sb = small.tile([128, c], f32)
    nc.vector.tensor_scalar_mul(out_sb, f2bc[:, 0, :], z_sb[:, 0:1])
    for i in range(1, c):
        nc.vector.scalar_tensor_tensor(out=out_sb, in0=f2bc[:, i, :],
                                       scalar=z_sb[:, i:i + 1], in1=out_sb,
                                       op0=OP.mult, op1=OP.add)
    if flags["f2_b"]:
        for j in range(c):
            nc.vector.tensor_single_scalar(out=out_sb[:, j:j + 1],
                                           in_=out_sb[:, j:j + 1],
                                           scalar=float(flags["f2b_vals"][j]),
                                           op=OP.add)
    nc.sync.dma_start(out=out_d, in_=out_sb)


def compile_kernel(cfg, flags, debug=False):
    """Build + compile; returns the Bass object ready for run_bass_kernel_spmd."""
    from contextlib import ExitStack

    import concourse.bacc as bacc
    import concourse.tile as tile

    nc = bacc.Bacc("TRN2", target_bir_lowering=False, debug=False)
    with tile.TileContext(nc) as tc:
        with ExitStack() as ctx:
            build_program(nc, tc, ctx, cfg, flags, debug=debug)
    nc.compile()
    return nc


def run(inputs, cfg=None, trace=False, debug=False):
    """Returns (full_output [B, C] f32, exec_time_ns or None)."""
    from concourse.bass_utils import run_bass_kernel_spmd

    if cfg is None:
        cfg = Cfg()
    shared, per_core, flags = host_prep(inputs, cfg)
    nc = compile_kernel(cfg, flags, debug=debug)
    in_maps = [{**shared, **pc} for pc in per_core]
    core_ids = list(range(len(in_maps)))
    res = run_bass_kernel_spmd(nc, in_maps, core_ids, trace=trace)
    out = np.concatenate([res.results[i]["out"] for i in core_ids], axis=0)
    if debug:
        return out, res.exec_time_ns, res
    return out, res.exec_time_ns


def kernel(**inputs) -> np.ndarray:
    out, _ = run(inputs)
    return out
